# revision 3
# baseline (speedup 1.0000x reference)
"""MAPE loss on 8 Trainium2 NeuronCores (raw Bass, software-pipelined).

MAPE = mean(|pred - label| / label) * 100 over 2**25 f32 elements.

Sharding: pure data parallel. Each of the 8 cores gets a contiguous 1/8
slice of both tensors (4,194,304 elements = 16 MiB per tensor per core,
32 MiB of HBM reads per core -> memory-bound, roofline ~94 us at
~358 GB/s per-NC HBM bandwidth).

Per core, per [128, F] f32 tile (pipelined, BUFS slots):
  sync (HWDGE): DMA x and y tiles into SBUF slot s
  ACT:          y <- Reciprocal(y)        (table act, ~1e-6 mean rel err)
  DVE:          x <- x * y                (tensor_tensor mult)
  ACT:          acc[:, i] = sum_f |x - 1| (Abs activation with accum_out)
Per-partition partial sums [128, NT] are DMA'd out per core; the final
mean is reduced on the host in float64.

|x/y - 1| == |x - y| / y exactly, since y > 0 (labels in (1e-3, 1)).

Raw Bass (not Tile): the Tile kernel-tail drain emits multi-wait CTRL
instructions this walrus build rejects ("Too many sync wait commands"),
and custom-DVE / ISA-class ops ("ISA wrong length") don't compile either.
Semaphore discipline:
  - sem_load[s] (one per buffer slot): +16 per DMA; tile k's loads are
    complete iff sem_load[k%B] >= 32*(k//B+1). Per-slot sems are needed
    because HWDGE completions across different tiles are not ordered.
  - rsem/msem/asem: recip/mult/abs completion counters (.then_inc on the
    instruction itself -- a separate sem_inc races with in-flight writes).
"""

import numpy as np

import concourse.bass as bass
from concourse import mybir
from concourse.bass_utils import run_bass_kernel_spmd

N_TOTAL = 33554432  # 2**25
N_CORES = 8
PER_CORE = N_TOTAL // N_CORES  # 4,194,304
P = 128  # SBUF partitions
F = 8192  # free-dim elements per tile
BUFS = 2  # SBUF buffer slots per stream (2 tensors * BUFS * F * 4B <= 128KB/part)
NT = PER_CORE // (P * F)  # DRAM tiles per core

AFT = mybir.ActivationFunctionType

# Results of the most recent run (BassKernelResults), for harness introspection.
last_results = None


def _act_reciprocal(nc, out_ap, in_ap):
    """InstActivation(func=Reciprocal) without the bass-level guard.

    The guard points at accuracy concerns; measured on this hardware the
    ACT reciprocal is ~1e-6 mean / ~1e-5 max relative error over the
    label range (1e-3, 1), far inside this problem's tolerance.
    Bias/scale/alpha must be immediates for Reciprocal (same as the
    public API's Copy/Reciprocal path).
    """
    ins = [nc.scalar.lower_ap(in_ap)]
    for v in (0.0, 1.0, 0.0):  # bias, scale, alpha
        ins.append(mybir.ImmediateValue(dtype=mybir.dt.float32, value=v))
    return nc.scalar.add_instruction(
        mybir.InstActivation(
            name=nc.get_next_instruction_name(),
            func=AFT.Reciprocal,
            ins=ins,
            outs=[nc.scalar.lower_ap(out_ap)],
        )
    )


def build_nc(F=F, BUFS=BUFS, R=1):
    """Build the per-core Bass program. R = in-NEFF repetition count
    (R>1 only for benchmarking; output is identical for any R)."""
    NT = PER_CORE // (P * F)
    T = R * NT
    nc = bass.Bass()
    x_h = nc.declare_dram_parameter(
        "predictions", [NT, P, F], mybir.dt.float32, isOutput=False
    )
    y_h = nc.declare_dram_parameter(
        "labels", [NT, P, F], mybir.dt.float32, isOutput=False
    )
    out_h = nc.declare_dram_parameter(
        "partials", [P, NT], mybir.dt.float32, isOutput=True
    )

    with (
        nc.sbuf_tensor([P, BUFS * F], mybir.dt.float32) as x_sb,
        nc.sbuf_tensor([P, BUFS * F], mybir.dt.float32) as y_sb,
        nc.sbuf_tensor([P, NT], mybir.dt.float32) as acc_sb,
        nc.sbuf_tensor([P, 1], mybir.dt.float32) as neg_sb,
        nc.semaphore() as rsem,
        nc.semaphore() as msem,
        nc.semaphore() as asem,
        nc.semaphore() as bsem,
        nc.semaphore() as osem,
    ):
        sem_ctxs = [nc.semaphore(f"sem_load{s}") for s in range(BUFS)]
        sem_load = [c.__enter__() for c in sem_ctxs]
        try:
            with nc.Block() as block:
                xs = lambda s: x_sb[:, s * F : (s + 1) * F]
                ys = lambda s: y_sb[:, s * F : (s + 1) * F]

                @block.sync
                def _(sync):
                    for k in range(T):
                        i, s = k % NT, k % BUFS
                        if k >= BUFS:
                            # slot free once abs of tile k-BUFS retired
                            sync.wait_ge(asem, k - BUFS + 1)
                        sync.dma_start(out=xs(s), in_=x_h[i]).then_inc(
                            sem_load[s], 16
                        )
                        sync.dma_start(out=ys(s), in_=y_h[i]).then_inc(
                            sem_load[s], 16
                        )
                    sync.wait_ge(asem, T)
                    sync.dma_start(out=out_h[:], in_=acc_sb[:]).then_inc(osem, 16)
                    sync.wait_ge(osem, 16)

                @block.vector
                def _(vector):
                    vector.memset(neg_sb[:], -1.0).then_inc(bsem, 1)
                    for k in range(T):
                        s = k % BUFS
                        vector.wait_ge(sem_load[s], 32 * (k // BUFS + 1))
                        vector.wait_ge(rsem, k + 1)
                        nc.vector.tensor_mul(xs(s), xs(s), ys(s)).then_inc(msem, 1)

                @block.scalar
                def _(scalar):
                    scalar.wait_ge(bsem, 1)

                    def do_abs(j):
                        sj, ij = j % BUFS, j % NT
                        scalar.wait_ge(msem, j + 1)
                        nc.scalar.activation(
                            out=xs(sj),
                            in_=xs(sj),
                            func=AFT.Abs,
                            bias=neg_sb[:, 0:1],
                            scale=1.0,
                            accum_out=acc_sb[:, ij : ij + 1],
                        ).then_inc(asem, 1)

                    for k in range(T):
                        s = k % BUFS
                        scalar.wait_ge(sem_load[s], 32 * (k // BUFS + 1))
                        _act_reciprocal(nc, ys(s), ys(s)).then_inc(rsem, 1)
                        if k >= 1:
                            do_abs(k - 1)
                    do_abs(T - 1)
        finally:
            for c in reversed(sem_ctxs):
                c.__exit__(None, None, None)
    return nc


def kernel(predictions, labels):
    global last_results
    preds = np.ascontiguousarray(np.asarray(predictions, dtype=np.float32)).reshape(
        N_CORES, NT, P, F
    )
    labs = np.ascontiguousarray(np.asarray(labels, dtype=np.float32)).reshape(
        N_CORES, NT, P, F
    )
    in_maps = [{"predictions": preds[c], "labels": labs[c]} for c in range(N_CORES)]
    nc = build_nc()
    last_results = run_bass_kernel_spmd(nc, in_maps, core_ids=list(range(N_CORES)))
    total = 0.0
    for r in last_results.results:
        total += r["partials"].astype(np.float64).sum()
    return np.float32(total / N_TOTAL * 100.0)
